# revision 1
# baseline (speedup 1.0000x reference)
"""Trainium2 Bass kernel for nn_Encoder (Keras-style LSTM encoder).

Reference computation (fp32):
    xz = einsum('btd,dg->btg', x, W) + b          # input projection
    per step t: z = xz[:, t] + h @ U
                i, f, g, o = split(z, 4); i,f,o=sigmoid; g=tanh
                c = f*c + i*g; h = o*tanh(c)
    returns (seq [B,T,UNITS], h_last [B,UNITS], c_last [B,UNITS])

Sharding: UNITS are sharded 8-ways (128 units/core); every core holds the
full batch (B=64).  Each core computes the 4x128 gate columns for its
units (z_c [64, 512]) by streaming its U/W column-slice through the PE
with h^T / x_t^T as the stationary operand, applies the pointwise cell
update for its units, and exchanges its h^T chunk [128, 64] with all
other cores via a per-step AllGather.  The time loop is fully unrolled
(collectives are not allowed inside control flow).

x is pre-transposed on the host to xT [T, D, B] so x_t^T chunks load as
contiguous stationary tiles; b is folded in as a rank-1 (K=1) matmul
with a ones vector; h^T is produced by a PE transpose (identity matmul).
"""
import os
import sys

sys.path.insert(0, "/opt/trn_rl_repo")

import numpy as np

B, T_FULL, D, UNITS = 64, 512, 256, 1024
N_CORES = 8
UC = UNITS // N_CORES          # 128 units per core
C = 4 * UC                     # 512 gate columns per core
KJ = UNITS // 128              # 8 contraction chunks for h @ U

_build_cache: dict = {}


def _build(t_steps: int):
    import concourse.bacc as bacc
    import concourse.tile as tile
    import concourse.mybir as mybir

    dt = mybir.dt.float32
    AF = mybir.ActivationFunctionType
    ALU = mybir.AluOpType

    nc = bacc.Bacc("TRN2", target_bir_lowering=False, debug=False,
                   num_devices=N_CORES)

    xT_ap = nc.dram_tensor("xT", [t_steps, D, B], dt, kind="ExternalInput").ap()
    Wc_ap = nc.dram_tensor("Wc", [D, C], dt, kind="ExternalInput").ap()
    Uc_ap = nc.dram_tensor("Uc", [UNITS, C], dt, kind="ExternalInput").ap()
    bc_ap = nc.dram_tensor("bc", [1, C], dt, kind="ExternalInput").ap()
    ones_ap = nc.dram_tensor("ones", [1, B], dt, kind="ExternalInput").ap()
    id_ap = nc.dram_tensor("iden", [B, B], dt, kind="ExternalInput").ap()
    seq_ap = nc.dram_tensor("seq", [t_steps, B, UC], dt, kind="ExternalOutput").ap()
    cl_ap = nc.dram_tensor("c_last", [B, UC], dt, kind="ExternalOutput").ap()

    with tile.TileContext(nc) as tc:
        with tc.tile_pool(name="const", bufs=1) as constp, \
             tc.tile_pool(name="xin", bufs=4) as xinp, \
             tc.tile_pool(name="gath", bufs=3) as gathp, \
             tc.tile_pool(name="state", bufs=2) as statep, \
             tc.tile_pool(name="work", bufs=3) as workp, \
             tc.tile_pool(name="psz", bufs=2, space="PSUM") as pszp, \
             tc.tile_pool(name="pst", bufs=2, space="PSUM") as pstp, \
             tc.tile_pool(name="dram", bufs=2, space="DRAM") as dramp:

            U_sb = constp.tile([128, KJ * C], dt, name="U_sb")
            for j in range(KJ):
                nc.sync.dma_start(U_sb[:, C * j:C * (j + 1)],
                                  Uc_ap[128 * j:128 * (j + 1), :])
            W_sb = constp.tile([128, 2 * C], dt, name="W_sb")
            for j in range(2):
                nc.sync.dma_start(W_sb[:, C * j:C * (j + 1)],
                                  Wc_ap[128 * j:128 * (j + 1), :])
            b_sb = constp.tile([1, C], dt, name="b_sb")
            nc.sync.dma_start(b_sb[:], bc_ap[:])
            ones_sb = constp.tile([1, B], dt, name="ones_sb")
            nc.sync.dma_start(ones_sb[:], ones_ap[:])
            id_sb = constp.tile([B, B], dt, name="id_sb")
            nc.sync.dma_start(id_sb[:], id_ap[:])

            hTg = gathp.tile([128, KJ * B], dt, name="hTg_init", tag="hTg")
            nc.vector.memset(hTg[:], 0.0)
            c_t = statep.tile([B, UC], dt, name="c_init", tag="c")
            nc.vector.memset(c_t[:], 0.0)

            for t in range(t_steps):
                xTt = xinp.tile([128, 2 * B], dt, name=f"xTt{t}", tag="xTt")
                nc.sync.dma_start(xTt[:, 0:B], xT_ap[t, 0:128, :])
                nc.sync.dma_start(xTt[:, B:2 * B], xT_ap[t, 128:256, :])

                z = pszp.tile([B, C], dt, name=f"z{t}", tag="z")
                nc.tensor.matmul(z[:], xTt[:, 0:B], W_sb[:, 0:C],
                                 start=True, stop=False)
                nc.tensor.matmul(z[:], xTt[:, B:2 * B], W_sb[:, C:2 * C],
                                 start=False, stop=False)
                nc.tensor.matmul(z[:], ones_sb[:], b_sb[:],
                                 start=False, stop=False)
                for j in range(KJ):
                    nc.tensor.matmul(z[:], hTg[:, B * j:B * (j + 1)],
                                     U_sb[:, C * j:C * (j + 1)],
                                     start=False, stop=(j == KJ - 1))

                # col layout within the core's slice: [i | f | o | g]
                sig = workp.tile([B, 3 * UC], dt, name=f"sig{t}", tag="sig")
                nc.scalar.activation(sig[:], z[:, 0:3 * UC], AF.Sigmoid)
                tg = workp.tile([B, UC], dt, name=f"tg{t}", tag="tg")
                nc.scalar.activation(tg[:], z[:, 3 * UC:4 * UC], AF.Tanh)

                ig = workp.tile([B, UC], dt, name=f"ig{t}", tag="ig")
                nc.vector.tensor_tensor(ig[:], sig[:, 0:UC], tg[:], ALU.mult)
                fc = workp.tile([B, UC], dt, name=f"fc{t}", tag="fc")
                nc.vector.tensor_tensor(fc[:], sig[:, UC:2 * UC], c_t[:],
                                        ALU.mult)
                c_t = statep.tile([B, UC], dt, name=f"c{t}", tag="c")
                nc.vector.tensor_add(c_t[:], ig[:], fc[:])
                tct = workp.tile([B, UC], dt, name=f"tct{t}", tag="tct")
                nc.scalar.activation(tct[:], c_t[:], AF.Tanh)
                h = workp.tile([B, UC], dt, name=f"h{t}", tag="h")
                nc.vector.tensor_tensor(h[:], sig[:, 2 * UC:3 * UC], tct[:],
                                        ALU.mult)

                nc.sync.dma_start(seq_ap[t], h[:])

                hT_ps = pstp.tile([UC, B], dt, name=f"hTps{t}", tag="hTps")
                nc.tensor.transpose(hT_ps[:], h[:], id_sb[:])
                hTs = workp.tile([UC, B], dt, name=f"hTs{t}", tag="hTs")
                nc.vector.tensor_copy(hTs[:], hT_ps[:])

                bin_ = dramp.tile([UC, B], dt, name=f"bin{t}", tag="bin")
                nc.sync.dma_start(bin_[:], hTs[:])
                bout = dramp.tile([UC * N_CORES, B], dt, name=f"bout{t}",
                                  tag="bout", addr_space="Shared")
                nc.gpsimd.collective_compute(
                    "AllGather", ALU.bypass,
                    replica_groups=[list(range(N_CORES))],
                    ins=[bin_.opt()], outs=[bout.opt()])
                hTg = gathp.tile([128, KJ * B], dt, name=f"hTg{t}", tag="hTg")
                for j in range(KJ):
                    nc.sync.dma_start(hTg[:, B * j:B * (j + 1)],
                                      bout[128 * j:128 * (j + 1), :])

            nc.sync.dma_start(cl_ap[:], c_t[:])

    nc.compile()
    return nc


def _get_nc(t_steps: int):
    if t_steps not in _build_cache:
        _build_cache[t_steps] = _build(t_steps)
    return _build_cache[t_steps]


def _make_in_maps(x, W, U, b, t_steps: int):
    xT = np.ascontiguousarray(np.transpose(x[:, :t_steps, :], (1, 2, 0)))
    ones = np.ones((1, B), np.float32)
    iden = np.eye(B, dtype=np.float32)
    in_maps = []
    for k in range(N_CORES):
        u0 = UC * k
        r = np.arange(u0, u0 + UC)
        # Keras kernel gate order is i|f|g|o; we lay the slice out as i|f|o|g
        idx = np.concatenate([r, UNITS + r, 3 * UNITS + r, 2 * UNITS + r])
        in_maps.append({
            "xT": xT,
            "Wc": np.ascontiguousarray(W[:, idx]),
            "Uc": np.ascontiguousarray(U[:, idx]),
            "bc": np.ascontiguousarray(b[idx])[None, :],
            "ones": ones,
            "iden": iden,
        })
    return in_maps


def kernel(x, W, U, b):
    t_steps = int(os.environ.get("LSTM_T_STEPS", T_FULL))
    from concourse import bass_utils

    nc = _get_nc(t_steps)
    in_maps = _make_in_maps(np.asarray(x, np.float32), np.asarray(W, np.float32),
                            np.asarray(U, np.float32), np.asarray(b, np.float32),
                            t_steps)
    res = bass_utils.run_bass_kernel_spmd(
        nc, in_maps, core_ids=list(range(N_CORES)))

    seq = np.empty((B, t_steps, UNITS), np.float32)
    c_last = np.empty((B, UNITS), np.float32)
    for k in range(N_CORES):
        seq[:, :, UC * k:UC * (k + 1)] = res.results[k]["seq"].transpose(1, 0, 2)
        c_last[:, UC * k:UC * (k + 1)] = res.results[k]["c_last"]
    h_last = np.ascontiguousarray(seq[:, -1, :])
    return seq, h_last, c_last


# revision 7
# speedup vs baseline: 100.1674x; 100.1674x over previous
"""Trainium2 Bass kernel for nn_Encoder (Keras-style LSTM encoder).

Reference computation (fp32):
    xz = einsum('btd,dg->btg', x, W) + b          # input projection
    per step t: z = xz[:, t] + h @ U
                i, f, g, o = split(z, 4); i,f,o=sigmoid; g=tanh
                c = f*c + i*g; h = o*tanh(c)
    returns (seq [B,T,UNITS], h_last [B,UNITS], c_last [B,UNITS])

Sharding: UNITS are sharded 8-ways (128 units/core); every core holds the
full batch (B=64).  Each core computes the 4x128 gate columns for its
units (z_c [64, 512]) by streaming its U/W column-slice through the PE
with h^T / x_t^T as the stationary operand, applies the pointwise cell
update for its units, and exchanges its h^T chunk [128, 64] with all
other cores via a per-step AllGather.  The time loop is fully unrolled
(collectives are not allowed inside control flow).

Matmul operands use float32r (same 4-byte encoding as fp32; the PE's
single-pass fp32 mode) — the 2x2-pass full-fp32 matmul path costs 4
cycles/row vs 1 for float32r at N>=256.

x is pre-transposed on the host to xT [T, D, B] so x_t^T chunks load as
contiguous stationary tiles; b is folded in as a rank-1 (K=1) matmul
with a ones vector; h^T is produced by a PE transpose (identity matmul).
"""
import os
import sys

sys.path.insert(0, "/opt/trn_rl_repo")

import numpy as np

B, T_FULL, D, UNITS = 64, 512, 256, 1024
N_CORES = 8
UC = UNITS // N_CORES          # 128 units per core
C = 4 * UC                     # 512 gate columns per core
KJ = UNITS // 128              # 8 contraction chunks for h @ U

_build_cache: dict = {}


def build(t_steps: int, io_t: int | None = None, do_ag: bool = True):
    """Build + bacc-compile the SPMD program.

    io_t: if set, xT/seq DRAM buffers hold io_t steps and step t uses
    slot t % io_t (benchmark mode that keeps host<->device IO small).
    """
    import concourse.bacc as bacc
    import concourse.tile as tile
    import concourse.mybir as mybir

    f32 = mybir.dt.float32
    f32r = mybir.dt.float32r
    AF = mybir.ActivationFunctionType
    ALU = mybir.AluOpType

    io_t = io_t or t_steps
    nc = bacc.Bacc("TRN2", target_bir_lowering=False, debug=False,
                   num_devices=N_CORES)

    xT_ap = nc.dram_tensor("xT", [io_t, D, B], f32, kind="ExternalInput").ap()
    Wc_ap = nc.dram_tensor("Wc", [D, C], f32, kind="ExternalInput").ap()
    Uc_ap = nc.dram_tensor("Uc", [UNITS, C], f32, kind="ExternalInput").ap()
    bc_ap = nc.dram_tensor("bc", [1, C], f32, kind="ExternalInput").ap()
    ones_ap = nc.dram_tensor("ones", [1, B], f32, kind="ExternalInput").ap()
    id_ap = nc.dram_tensor("iden", [B, B], f32, kind="ExternalInput").ap()
    hz_ap = nc.dram_tensor("hzero", [128, KJ * B], f32, kind="ExternalInput").ap()
    seq_ap = nc.dram_tensor("seq", [io_t, B, UC], f32, kind="ExternalOutput").ap()
    cl_ap = nc.dram_tensor("c_last", [B, UC], f32, kind="ExternalOutput").ap()

    with tile.TileContext(nc) as tc:
        with tc.tile_pool(name="const", bufs=1) as constp, \
             tc.tile_pool(name="xin", bufs=4) as xinp, \
             tc.tile_pool(name="gath", bufs=3) as gathp, \
             tc.tile_pool(name="state", bufs=2) as statep, \
             tc.tile_pool(name="work", bufs=3) as workp, \
             tc.tile_pool(name="psz", bufs=2, space="PSUM") as pszp, \
             tc.tile_pool(name="pst", bufs=2, space="PSUM") as pstp, \
             tc.tile_pool(name="dram", bufs=2, space="DRAM") as dramp:

            U_sb = constp.tile([128, KJ * C], f32r, name="U_sb")
            for j in range(KJ):
                nc.sync.dma_start(U_sb[:, C * j:C * (j + 1)],
                                  Uc_ap[128 * j:128 * (j + 1), :].bitcast(f32r))
            W_sb = constp.tile([128, 2 * C], f32r, name="W_sb")
            for j in range(2):
                nc.sync.dma_start(W_sb[:, C * j:C * (j + 1)],
                                  Wc_ap[128 * j:128 * (j + 1), :].bitcast(f32r))
            b_sb = constp.tile([1, C], f32r, name="b_sb")
            nc.sync.dma_start(b_sb[:], bc_ap[:].bitcast(f32r))
            ones_sb = constp.tile([1, B], f32r, name="ones_sb")
            nc.sync.dma_start(ones_sb[:], ones_ap[:].bitcast(f32r))
            id_sb = constp.tile([B, B], f32, name="id_sb")
            nc.sync.dma_start(id_sb[:], id_ap[:])

            hTg = gathp.tile([128, KJ * B], f32r, name="hTg_init", tag="hTg")
            nc.sync.dma_start(hTg[:], hz_ap[:].bitcast(f32r))
            c_t = statep.tile([B, UC], f32, name="c_init", tag="c")
            nc.vector.memset(c_t[:], 0.0)

            for t in range(t_steps):
                ti = t % io_t
                xTt = xinp.tile([128, 2 * B], f32r, name=f"xTt{t}", tag="xTt")
                nc.sync.dma_start(xTt[:, 0:B], xT_ap[ti, 0:128, :].bitcast(f32r))
                nc.sync.dma_start(xTt[:, B:2 * B], xT_ap[ti, 128:256, :].bitcast(f32r))

                z = pszp.tile([B, C], f32, name=f"z{t}", tag="z")
                nc.tensor.matmul(z[:], xTt[:, 0:B], W_sb[:, 0:C],
                                 start=True, stop=False)
                nc.tensor.matmul(z[:], xTt[:, B:2 * B], W_sb[:, C:2 * C],
                                 start=False, stop=False)
                nc.tensor.matmul(z[:], ones_sb[:], b_sb[:],
                                 start=False, stop=False)
                for j in range(KJ):
                    nc.tensor.matmul(z[:], hTg[:, B * j:B * (j + 1)],
                                     U_sb[:, C * j:C * (j + 1)],
                                     start=False, stop=(j == KJ - 1))

                # col layout within the core's slice: [i | f | o | g]
                sig = workp.tile([B, 3 * UC], f32, name=f"sig{t}", tag="sig")
                nc.scalar.activation(sig[:], z[:, 0:3 * UC], AF.Sigmoid)
                tg = workp.tile([B, UC], f32, name=f"tg{t}", tag="tg")
                nc.scalar.activation(tg[:], z[:, 3 * UC:4 * UC], AF.Tanh)

                ig = workp.tile([B, UC], f32, name=f"ig{t}", tag="ig")
                nc.vector.tensor_tensor(ig[:], sig[:, 0:UC], tg[:], ALU.mult)
                fc = workp.tile([B, UC], f32, name=f"fc{t}", tag="fc")
                nc.vector.tensor_tensor(fc[:], sig[:, UC:2 * UC], c_t[:],
                                        ALU.mult)
                c_t = statep.tile([B, UC], f32, name=f"c{t}", tag="c")
                nc.vector.tensor_add(c_t[:], ig[:], fc[:])
                tct = workp.tile([B, UC], f32, name=f"tct{t}", tag="tct")
                nc.scalar.activation(tct[:], c_t[:], AF.Tanh)
                h = workp.tile([B, UC], f32, name=f"h{t}", tag="h")
                nc.vector.tensor_tensor(h[:], sig[:, 2 * UC:3 * UC], tct[:],
                                        ALU.mult)

                nc.gpsimd.dma_start(seq_ap[ti], h[:])

                hT_ps = pstp.tile([UC, B], f32, name=f"hTps{t}", tag="hTps")
                nc.tensor.transpose(hT_ps[:], h[:], id_sb[:])
                hTs = workp.tile([UC, B], f32r, name=f"hTs{t}", tag="hTs")
                nc.vector.tensor_copy(hTs[:], hT_ps[:])

                if do_ag:
                    bin_ = dramp.tile([UC, B], f32, name=f"bin{t}", tag="bin")
                    nc.scalar.dma_start(bin_[:].bitcast(f32r), hTs[:])
                    bout = dramp.tile([UC * N_CORES, B], f32, name=f"bout{t}",
                                      tag="bout", addr_space="Shared")
                    nc.gpsimd.collective_compute(
                        "AllGather", ALU.bypass,
                        replica_groups=[list(range(N_CORES))],
                        ins=[bin_.opt()], outs=[bout.opt()])
                    hTg = gathp.tile([128, KJ * B], f32r, name=f"hTg{t}",
                                     tag="hTg")
                    nc.scalar.dma_start(
                        hTg[:].rearrange("p (j b) -> p j b", j=KJ),
                        bout[:].bitcast(f32r).rearrange("(j p) b -> p j b", p=128))
                else:
                    hTg = gathp.tile([128, KJ * B], f32r, name=f"hTg{t}",
                                     tag="hTg")
                    nc.vector.tensor_copy(hTg[:, 0:B], hTs[:])

            nc.gpsimd.dma_start(cl_ap[:], c_t[:])

    nc.compile()
    return nc


def _get_nc(t_steps: int):
    if t_steps not in _build_cache:
        _build_cache[t_steps] = build(t_steps)
    return _build_cache[t_steps]


def _make_in_maps(x, W, U, b, t_steps: int):
    xT = np.ascontiguousarray(np.transpose(x[:, :t_steps, :], (1, 2, 0)))
    ones = np.ones((1, B), np.float32)
    iden = np.eye(B, dtype=np.float32)
    in_maps = []
    for k in range(N_CORES):
        u0 = UC * k
        r = np.arange(u0, u0 + UC)
        # Keras kernel gate order is i|f|g|o; we lay the slice out as i|f|o|g
        idx = np.concatenate([r, UNITS + r, 3 * UNITS + r, 2 * UNITS + r])
        in_maps.append({
            "xT": xT,
            "Wc": np.ascontiguousarray(W[:, idx]),
            "Uc": np.ascontiguousarray(U[:, idx]),
            "bc": np.ascontiguousarray(b[idx])[None, :],
            "ones": ones,
            "iden": iden,
            "hzero": np.zeros((128, KJ * B), np.float32),
        })
    return in_maps


def kernel(x, W, U, b):
    t_steps = int(os.environ.get("LSTM_T_STEPS", T_FULL))
    from concourse import bass_utils

    nc = _get_nc(t_steps)
    in_maps = _make_in_maps(np.asarray(x, np.float32), np.asarray(W, np.float32),
                            np.asarray(U, np.float32), np.asarray(b, np.float32),
                            t_steps)
    res = bass_utils.run_bass_kernel_spmd(
        nc, in_maps, core_ids=list(range(N_CORES)))

    seq = np.empty((B, t_steps, UNITS), np.float32)
    c_last = np.empty((B, UNITS), np.float32)
    for k in range(N_CORES):
        seq[:, :, UC * k:UC * (k + 1)] = res.results[k]["seq"].transpose(1, 0, 2)
        c_last[:, UC * k:UC * (k + 1)] = res.results[k]["c_last"]
    h_last = np.ascontiguousarray(seq[:, -1, :])
    return seq, h_last, c_last
